# revision 4
# baseline (speedup 1.0000x reference)
"""GCN (2-layer, symmetric-norm, self-loops) on 8 TRN2 NeuronCores.

Strategy:
  - Nodes dst-sharded across 8 cores (12500 each, padded to 12544 = 98*128).
  - Per conv: h@W computed per core; g = dinv*h AllGathered into a full
    replicated table [100352, 64] f32 in each core's DRAM.
  - Per-edge message gather via gpsimd dma_gather (int16 indices -> 4
    source ranges of 25088 rows), edge-major groups of 128.
  - Segment-sum via one-hot matmul: onehot[128 edges, 128 dst] built on DVE
    (iota vs dst_local compare), accumulated in PSUM per 128-node window.
  - Self-loop term, bias, relu, next-layer matmuls, log_softmax on device.
  - Host does schedule building only: edge sorting/padding, gather index
    construction, degree bincount for normalization, input layout.
  - The group structure (counts per window/range) is max-pooled over cores
    so all 8 cores run the same SPMD program; per-core data (indices,
    dst-locals, dinv, x) differ via in_maps.
"""

import os
import sys
import numpy as np

sys.path.insert(0, "/opt/trn_rl_repo")

N = 100000
NC = 8
PER = 12500          # real nodes per core
PERP = 12544         # padded (98 * 128)
NW = PERP // 128     # 98 windows
V = NC * PERP        # 100352 table rows
NRANGE = 4
RSZ = V // NRANGE    # 25088 (= 2 shards), < 32768 so int16 works
F_IN = 128
HID = 64
N_CLS = 40
SBW = 4              # windows per superblock (gather call granularity)

_cache = {}


def _build_schedule(src, dst):
    """Host schedule: uniform-across-cores group structure, per-core data."""
    g_row = (src // PER) * PERP + (src % PER)   # gather-table row of src node
    core = dst // PER
    dstl = dst - core * PER                     # local dst 0..12499
    w = dstl // 128
    r = g_row // RSZ
    loc = (g_row % RSZ).astype(np.int16)        # in-range row, 0..25087
    wloc = (dstl % 128).astype(np.float32)

    key = (core * NW + w) * NRANGE + r
    order = np.argsort(key, kind="stable")
    key_s = key[order]
    loc_s = loc[order]
    wloc_s = wloc[order]
    cell_ids, cell_counts = np.unique(key_s, return_counts=True)
    counts_flat = np.zeros(NC * NW * NRANGE, np.int64)
    counts_flat[cell_ids] = cell_counts
    counts = counts_flat.reshape(NC, NW, NRANGE)

    # uniform group counts: max over cores per (w, r)
    J_wr = np.ceil(counts.max(axis=0) / 128).astype(np.int64)   # [NW, NRANGE]
    J_wr = np.maximum(J_wr, 1)
    NG = int(J_wr.sum())

    dummy_loc = np.int16(PER)   # per-range zero pad row (shard-local row 12500)

    cell_starts = np.zeros(NC * NW * NRANGE + 1, np.int64)
    np.cumsum(counts_flat, out=cell_starts[1:])

    idx_streams = []
    dstloc_cols = []
    for c in range(NC):
        idx_parts = []
        dl_parts = []
        for wi in range(NW):
            for ri in range(NRANGE):
                cid = (c * NW + wi) * NRANGE + ri
                s0, s1 = cell_starts[cid], cell_starts[cid + 1]
                n_real = s1 - s0
                n_pad = int(J_wr[wi, ri]) * 128
                li = np.full(n_pad, dummy_loc, np.int16)
                li[:n_real] = loc_s[s0:s1]
                dl = np.zeros(n_pad, np.float32)
                dl[:n_real] = wloc_s[s0:s1]
                idx_parts.append(li)
                dl_parts.append(dl)
        idx_all = np.concatenate(idx_parts)
        dl_all = np.concatenate(dl_parts)
        dstloc_cols.append(np.ascontiguousarray(dl_all.reshape(NG, 128).T))
        idx_streams.append(idx_all)

    # group-index offsets in (w, r, j) order
    g_off_wr = np.zeros((NW, NRANGE), np.int64)
    t = 0
    for wi in range(NW):
        for ri in range(NRANGE):
            g_off_wr[wi, ri] = t
            t += int(J_wr[wi, ri])

    # gather calls: per (superblock, range)
    n_sb = (NW + SBW - 1) // SBW
    call_meta = []   # (nrows, free16_off, range, [(gidx, window, col_in_call)])
    foff = 0
    for sb in range(n_sb):
        ws = range(sb * SBW, min((sb + 1) * SBW, NW))
        for ri in range(NRANGE):
            groups = []
            jj = 0
            for wi in ws:
                for j in range(int(J_wr[wi, ri])):
                    groups.append((int(g_off_wr[wi, ri] + j), wi, jj))
                    jj += 1
            if jj:
                call_meta.append((jj * 128, foff, ri, groups))
                foff += jj * 8   # (jj*128)/16

    # per-core wrapped int16 idx streams in call order
    idx_wrapped = []
    for c in range(NC):
        stream = idx_streams[c]
        parts = []
        for nrows, _foff, _ri, groups in call_meta:
            rows = np.concatenate(
                [stream[g * 128:(g + 1) * 128] for (g, _, _) in groups])
            blk = rows.reshape(nrows // 16, 16).T           # [16, nrows/16]
            parts.append(np.tile(blk, (8, 1)))              # [128, nrows/16]
        idx_wrapped.append(np.ascontiguousarray(np.concatenate(parts, 1)))

    gpw = [int(J_wr[wi].sum()) for wi in range(NW)]
    return {
        "J_wr": J_wr, "NG": NG, "call_meta": call_meta, "gpw": gpw,
        "idx_wrapped": idx_wrapped, "dstloc": dstloc_cols, "total_f16": foff,
    }


def _build_program(sched):
    import concourse.bacc as bacc
    import concourse.tile as tile
    import concourse.mybir as mybir
    from concourse.masks import make_identity

    f32 = mybir.dt.float32
    i16 = mybir.dt.int16
    i32 = mybir.dt.int32
    AF = mybir.ActivationFunctionType
    OP = mybir.AluOpType
    NG = sched["NG"]
    call_meta = sched["call_meta"]
    GPW = sched["gpw"]
    TOT16 = sched["total_f16"]

    nc = bacc.Bacc("TRN2", target_bir_lowering=False, debug=False,
                   num_devices=NC, num_swdge_queues=4)

    xT = nc.dram_tensor("xT", [F_IN, PERP], f32, kind="ExternalInput")
    idx = nc.dram_tensor("idx", [128, TOT16], i16, kind="ExternalInput")
    dstloc = nc.dram_tensor("dstloc", [128, NG], f32, kind="ExternalInput")
    dinv_c = nc.dram_tensor("dinv_c", [128, NW], f32, kind="ExternalInput")
    e_meta = nc.dram_tensor("e_meta", [1, F_IN], f32, kind="ExternalInput")
    w0r = nc.dram_tensor("w0r", [1, F_IN], f32, kind="ExternalInput")
    b0T = nc.dram_tensor("b0T", [F_IN, F_IN], f32, kind="ExternalInput")
    W1 = nc.dram_tensor("W1", [F_IN, HID], f32, kind="ExternalInput")
    W2 = nc.dram_tensor("W2", [HID, HID], f32, kind="ExternalInput")
    Wl = nc.dram_tensor("Wl", [HID, N_CLS], f32, kind="ExternalInput")
    bc1b = nc.dram_tensor("bc1b", [128, HID], f32, kind="ExternalInput")
    bc2b = nc.dram_tensor("bc2b", [128, HID], f32, kind="ExternalInput")
    blb = nc.dram_tensor("blb", [128, N_CLS], f32, kind="ExternalInput")
    padm = nc.dram_tensor("padm", [128, 1], f32, kind="ExternalInput")
    out = nc.dram_tensor("out", [PERP, N_CLS], f32, kind="ExternalOutput")

    with tile.TileContext(nc) as tc:
        with (
            tc.tile_pool(name="dram", bufs=1, space="DRAM") as dram,
            tc.tile_pool(name="consts", bufs=1) as consts,
            tc.tile_pool(name="persist", bufs=1) as persist,
            tc.tile_pool(name="xs", bufs=3) as xs,
            tc.tile_pool(name="gath", bufs=6) as gpool,
            tc.tile_pool(name="oh", bufs=4) as ohp,
            tc.tile_pool(name="post", bufs=3) as post,
            tc.tile_pool(name="ps", bufs=3, space="PSUM") as ps,
            tc.tile_pool(name="ps2", bufs=3, space="PSUM") as ps2,
        ):
            ident = consts.tile([128, 128], f32)
            make_identity(nc, ident[:])
            ident64 = consts.tile([64, 64], f32)
            make_identity(nc, ident64[:])
            iota_i = consts.tile([128, 128], i32)
            nc.gpsimd.iota(iota_i[:], pattern=[[1, 128]], base=0,
                           channel_multiplier=0)
            iota_t = consts.tile([128, 128], f32)
            nc.vector.tensor_copy(iota_t[:], iota_i[:])

            dinv_t = consts.tile([128, NW], f32)
            nc.sync.dma_start(dinv_t[:], dinv_c[:])
            dstloc_t = consts.tile([128, NG], f32)
            nc.sync.dma_start(dstloc_t[:], dstloc[:])
            bc1_t = consts.tile([128, HID], f32)
            nc.sync.dma_start(bc1_t[:], bc1b[:])
            bc2_t = consts.tile([128, HID], f32)
            nc.sync.dma_start(bc2_t[:], bc2b[:])
            bl_t = consts.tile([128, N_CLS], f32)
            nc.sync.dma_start(bl_t[:], blb[:])
            padm_t = consts.tile([128, 1], f32)
            nc.sync.dma_start(padm_t[:], padm[:])
            em_t = consts.tile([1, F_IN], f32)
            nc.sync.dma_start(em_t[:], e_meta[:])
            w0_t = consts.tile([1, F_IN], f32)
            nc.sync.dma_start(w0_t[:], w0r[:])
            b0T_t = consts.tile([F_IN, F_IN], f32)
            nc.sync.dma_start(b0T_t[:], b0T[:])
            W1_t = consts.tile([F_IN, HID], f32)
            nc.sync.dma_start(W1_t[:], W1[:])
            W2_t = consts.tile([HID, HID], f32)
            nc.sync.dma_start(W2_t[:], W2[:])
            Wl_t = consts.tile([HID, N_CLS], f32)
            nc.sync.dma_start(Wl_t[:], Wl[:])

            # stage 0: W01[f_in, hid] = relu(w0@E_meta + b0) @ W1
            w0p = ps2.tile([128, 128], f32, tag="t")
            nc.tensor.matmul(w0p[:], lhsT=em_t[:], rhs=w0_t[:],
                             start=True, stop=True)
            w0s = consts.tile([128, 128], f32)
            nc.vector.tensor_add(w0s[:], w0p[:], b0T_t[:])
            nc.scalar.activation(w0s[:], w0s[:], AF.Relu)
            w01p = ps2.tile([128, HID], f32, tag="t")
            nc.tensor.matmul(w01p[:], lhsT=w0s[:], rhs=W1_t[:],
                             start=True, stop=True)
            W01_t = consts.tile([128, HID], f32)
            nc.vector.tensor_copy(W01_t[:], w01p[:])

            g1b = dram.tile([PERP, HID], f32)
            g2b = dram.tile([PERP, HID], f32)
            G1 = dram.tile([V, HID], f32)
            G2 = dram.tile([V, HID], f32)

            # stage 1: g1 (node-major) = dinv * (x @ W01), per window
            g1w = []
            for w in range(NW):
                xw = xs.tile([128, 128], f32, tag="xw")
                nc.sync.dma_start(xw[:], xT[:, w * 128:(w + 1) * 128])
                h1p = ps2.tile([HID, 128], f32, tag="t")
                nc.tensor.matmul(h1p[:], lhsT=W01_t[:], rhs=xw[:],
                                 start=True, stop=True)
                h1s = xs.tile([HID, 128], f32, tag="h1s")
                nc.vector.tensor_copy(h1s[:], h1p[:])
                h1tp = ps2.tile([128, HID], f32, tag="t")
                nc.tensor.transpose(out=h1tp[:], in_=h1s[:], identity=ident64[:])
                gw = persist.tile([128, HID], f32, tag=f"g1_{w}")
                nc.vector.tensor_scalar_mul(gw[:], h1tp[:], dinv_t[:, w:w + 1])
                nc.sync.dma_start(g1b[w * 128:(w + 1) * 128, :], gw[:])
                g1w.append(gw)

            nc.gpsimd.collective_compute(
                "AllGather", mybir.AluOpType.bypass,
                replica_groups=[list(range(NC))],
                ins=[g1b.opt()], outs=[G1.opt()],
            )

            def conv_pass(Gtab, gself, bc_t, out_hook):
                ci = 0
                win_groups = [[] for _ in range(NW)]
                for nrows, foff, ri, groups in call_meta:
                    J = nrows // 128
                    idxt = xs.tile([128, nrows // 16], i16, tag="idxt")
                    nc.sync.dma_start(idxt[:], idx[:, foff:foff + nrows // 16])
                    gt = gpool.tile([128, J, HID], f32, tag="gt")
                    nc.gpsimd.dma_gather(
                        out_ap=gt[:],
                        in_ap=Gtab[ri * RSZ:(ri + 1) * RSZ, :],
                        idxs_ap=idxt[:],
                        num_idxs=nrows,
                        num_idxs_reg=nrows,
                        elem_size=HID,
                        single_packet=False,
                        queue_num=ci % 4,
                    )
                    ci += 1
                    for (gidx, wi, jj) in groups:
                        win_groups[wi].append((gt, jj, gidx))
                        if len(win_groups[wi]) == GPW[wi]:
                            pw = ps.tile([128, HID], f32, tag="seg")
                            ng = len(win_groups[wi])
                            for k, (gt_, jj_, gidx_) in enumerate(win_groups[wi]):
                                oh = ohp.tile([128, 128], f32, tag="oh")
                                nc.vector.tensor_scalar(
                                    out=oh[:], in0=iota_t[:],
                                    scalar1=dstloc_t[:, gidx_:gidx_ + 1],
                                    scalar2=None, op0=OP.is_equal,
                                )
                                nc.tensor.matmul(
                                    pw[:], lhsT=oh[:], rhs=gt_[:, jj_, :],
                                    start=(k == 0), stop=(k == ng - 1),
                                )
                            out_hook(wi, pw, gself[wi], bc_t)
                            win_groups[wi] = []

            g2w = []

            def post1(wi, pw, gsw, bct):
                t1 = post.tile([128, HID], f32, tag="t1")
                nc.vector.tensor_add(t1[:], pw[:], gsw[:])
                nc.vector.tensor_scalar_mul(t1[:], t1[:], dinv_t[:, wi:wi + 1])
                nc.vector.tensor_add(t1[:], t1[:], bct[:])
                nc.scalar.activation(t1[:], t1[:], AF.Relu)
                if wi == NW - 1:
                    nc.vector.tensor_scalar_mul(t1[:], t1[:], padm_t[:, 0:1])
                atp = ps2.tile([HID, 128], f32, tag="t")
                nc.tensor.transpose(out=atp[:], in_=t1[:], identity=ident[:])
                ats = post.tile([HID, 128], f32, tag="a1t")
                nc.vector.tensor_copy(ats[:], atp[:])
                h2p = ps2.tile([128, HID], f32, tag="t")
                nc.tensor.matmul(h2p[:], lhsT=ats[:], rhs=W2_t[:],
                                 start=True, stop=True)
                gw2 = persist.tile([128, HID], f32, tag=f"g2_{wi}")
                nc.vector.tensor_scalar_mul(gw2[:], h2p[:], dinv_t[:, wi:wi + 1])
                nc.sync.dma_start(g2b[wi * 128:(wi + 1) * 128, :], gw2[:])
                g2w.append(gw2)

            conv_pass(G1, g1w, bc1_t, post1)

            nc.gpsimd.collective_compute(
                "AllGather", mybir.AluOpType.bypass,
                replica_groups=[list(range(NC))],
                ins=[g2b.opt()], outs=[G2.opt()],
            )

            def post2(wi, pw, gsw, bct):
                t1 = post.tile([128, HID], f32, tag="t2")
                nc.vector.tensor_add(t1[:], pw[:], gsw[:])
                nc.vector.tensor_scalar_mul(t1[:], t1[:], dinv_t[:, wi:wi + 1])
                nc.vector.tensor_add(t1[:], t1[:], bct[:])
                nc.scalar.activation(t1[:], t1[:], AF.Relu)
                atp = ps2.tile([HID, 128], f32, tag="t")
                nc.tensor.transpose(out=atp[:], in_=t1[:], identity=ident[:])
                ats = post.tile([HID, 128], f32, tag="a2t")
                nc.vector.tensor_copy(ats[:], atp[:])
                lp = ps2.tile([128, N_CLS], f32, tag="t")
                nc.tensor.matmul(lp[:], lhsT=ats[:], rhs=Wl_t[:],
                                 start=True, stop=True)
                lg = post.tile([128, N_CLS], f32, tag="lgs")
                nc.vector.tensor_add(lg[:], lp[:], bl_t[:])
                mx = post.tile([128, 1], f32, tag="mx")
                nc.vector.tensor_reduce(out=mx[:], in_=lg[:],
                                        axis=mybir.AxisListType.X, op=OP.max)
                nc.vector.tensor_scalar(out=lg[:], in0=lg[:], scalar1=mx[:],
                                        scalar2=None, op0=OP.subtract)
                ex = post.tile([128, N_CLS], f32, tag="ex")
                nc.scalar.activation(ex[:], lg[:], AF.Exp)
                sm = post.tile([128, 1], f32, tag="sm")
                nc.vector.tensor_reduce(out=sm[:], in_=ex[:],
                                        axis=mybir.AxisListType.X, op=OP.add)
                nc.scalar.activation(sm[:], sm[:], AF.Ln)
                nc.vector.tensor_scalar(out=lg[:], in0=lg[:], scalar1=sm[:],
                                        scalar2=None, op0=OP.subtract)
                nc.sync.dma_start(out[wi * 128:(wi + 1) * 128, :], lg[:])

            conv_pass(G2, g2w, bc2_t, post2)

    nc.compile()
    return nc


def kernel(**inputs):
    from concourse import bass_utils

    x = np.asarray(inputs["x"], np.float32)
    ei = np.asarray(inputs["edge_index"])
    E_meta = np.asarray(inputs["E_meta"], np.float32)
    w0 = np.asarray(inputs["w0"], np.float32)
    b0 = np.asarray(inputs["b0"], np.float32)
    W1 = np.asarray(inputs["W1"], np.float32)
    bc1 = np.asarray(inputs["bc1"], np.float32)
    W2 = np.asarray(inputs["W2"], np.float32)
    bc2 = np.asarray(inputs["bc2"], np.float32)
    Wl = np.asarray(inputs["Wl"], np.float32)
    bl = np.asarray(inputs["bl"], np.float32)

    src = ei[0].astype(np.int64)
    dst = ei[1].astype(np.int64)

    sched = _build_schedule(src, dst)
    key = (sched["NG"], sched["total_f16"],
           tuple(int(v) for v in sched["J_wr"].flatten()))
    if key not in _cache:
        _cache[key] = _build_program(sched)
    nc = _cache[key]

    deg = 1.0 + np.bincount(dst, minlength=N).astype(np.float32)
    dinv = (1.0 / np.sqrt(deg)).astype(np.float32)

    in_maps = []
    for c in range(NC):
        xTc = np.zeros((F_IN, PERP), np.float32)
        xTc[:, :PER] = x[c * PER:(c + 1) * PER].T
        dv = np.ones(PERP, np.float32)
        dv[:PER] = dinv[c * PER:(c + 1) * PER]
        in_maps.append({
            "xT": xTc,
            "idx": sched["idx_wrapped"][c],
            "dstloc": sched["dstloc"][c],
            "dinv_c": np.ascontiguousarray(dv.reshape(NW, 128).T),
            "e_meta": E_meta,
            "w0r": np.ascontiguousarray(w0.reshape(1, F_IN)),
            "b0T": np.ascontiguousarray(b0.T),
            "W1": W1, "W2": W2, "Wl": Wl,
            "bc1b": np.tile(bc1.reshape(1, HID), (128, 1)),
            "bc2b": np.tile(bc2.reshape(1, HID), (128, 1)),
            "blb": np.tile(bl.reshape(1, N_CLS), (128, 1)),
            "padm": np.concatenate([np.ones((84, 1), np.float32),
                                    np.zeros((44, 1), np.float32)]),
        })

    res = bass_utils.run_bass_kernel_spmd(
        nc, in_maps, core_ids=list(range(NC)),
        trace=bool(int(os.environ.get("GCN_TRACE", "0"))),
    )
    kernel.last_exec_time_ns = res.exec_time_ns

    outp = np.empty((N, N_CLS), np.float32)
    for c in range(NC):
        outp[c * PER:(c + 1) * PER] = res.results[c]["out"][:PER]
    return outp


# revision 5
# speedup vs baseline: 1.4252x; 1.4252x over previous
"""GCN (2-layer, symmetric-norm, self-loops) on 8 TRN2 NeuronCores.

Strategy:
  - Nodes dst-sharded across 8 cores (12500 each, padded to 12544 = 98*128).
  - Per conv: h@W computed per core; g = dinv*h AllGathered into a full
    replicated table [100352, 64] f32 in each core's DRAM.
  - Per-edge message gather via gpsimd dma_gather (int16 indices -> 4
    source ranges of 25088 rows), edge-major groups of 128.
  - Segment-sum via one-hot matmul: onehot[128 edges, 128 dst] built on DVE
    (iota vs dst_local compare), accumulated in PSUM per 128-node window.
  - Self-loop term, bias, relu, next-layer matmuls, log_softmax on device.
  - Host does schedule building only: edge sorting/padding, gather index
    construction, degree bincount for normalization, input layout.
  - The group structure (counts per window/range) is max-pooled over cores
    so all 8 cores run the same SPMD program; per-core data (indices,
    dst-locals, dinv, x) differ via in_maps.
"""

import os
import sys
import numpy as np

sys.path.insert(0, "/opt/trn_rl_repo")

N = 100000
NC = 8
PER = 12500          # real nodes per core
PERP = 12544         # padded (98 * 128)
NW = PERP // 128     # 98 windows
V = NC * PERP        # 100352 table rows
NRANGE = 4
RSZ = V // NRANGE    # 25088 (= 2 shards), < 32768 so int16 works
F_IN = 128
HID = 64
N_CLS = 40
SBW = 4              # windows per superblock (gather call granularity)

_cache = {}


def _iotab():
    import ml_dtypes
    return np.tile(np.arange(128, dtype=np.float32), (128, 1)).astype(
        ml_dtypes.bfloat16)


def _build_schedule(src, dst):
    """Host schedule: uniform-across-cores group structure, per-core data."""
    g_row = (src // PER) * PERP + (src % PER)   # gather-table row of src node
    core = dst // PER
    dstl = dst - core * PER                     # local dst 0..12499
    w = dstl // 128
    r = g_row // RSZ
    loc = (g_row % RSZ).astype(np.int16)        # in-range row, 0..25087
    wloc = (dstl % 128).astype(np.float32)

    key = (core * NW + w) * NRANGE + r
    order = np.argsort(key, kind="stable")
    key_s = key[order]
    loc_s = loc[order]
    wloc_s = wloc[order]
    cell_ids, cell_counts = np.unique(key_s, return_counts=True)
    counts_flat = np.zeros(NC * NW * NRANGE, np.int64)
    counts_flat[cell_ids] = cell_counts
    counts = counts_flat.reshape(NC, NW, NRANGE)

    # uniform group counts: max over cores per (w, r)
    J_wr = np.ceil(counts.max(axis=0) / 128).astype(np.int64)   # [NW, NRANGE]
    J_wr = np.maximum(J_wr, 1)
    NG = int(J_wr.sum())

    dummy_loc = np.int16(PER)   # per-range zero pad row (shard-local row 12500)

    cell_starts = np.zeros(NC * NW * NRANGE + 1, np.int64)
    np.cumsum(counts_flat, out=cell_starts[1:])

    idx_streams = []
    dstloc_cols = []
    for c in range(NC):
        idx_parts = []
        dl_parts = []
        for wi in range(NW):
            for ri in range(NRANGE):
                cid = (c * NW + wi) * NRANGE + ri
                s0, s1 = cell_starts[cid], cell_starts[cid + 1]
                n_real = s1 - s0
                n_pad = int(J_wr[wi, ri]) * 128
                li = np.full(n_pad, dummy_loc, np.int16)
                li[:n_real] = loc_s[s0:s1]
                dl = np.zeros(n_pad, np.float32)
                dl[:n_real] = wloc_s[s0:s1]
                idx_parts.append(li)
                dl_parts.append(dl)
        idx_all = np.concatenate(idx_parts)
        dl_all = np.concatenate(dl_parts)
        dstloc_cols.append(dl_all.reshape(NG, 128).T)   # (w,r,j) order for now
        idx_streams.append(idx_all)

    # group-index offsets in (w, r, j) order
    g_off_wr = np.zeros((NW, NRANGE), np.int64)
    t = 0
    for wi in range(NW):
        for ri in range(NRANGE):
            g_off_wr[wi, ri] = t
            t += int(J_wr[wi, ri])

    # gather calls: per (superblock, range)
    n_sb = (NW + SBW - 1) // SBW
    call_meta = []   # (nrows, free16_off, range, [(gidx, window, col_in_call)])
    foff = 0
    for sb in range(n_sb):
        ws = range(sb * SBW, min((sb + 1) * SBW, NW))
        for ri in range(NRANGE):
            groups = []
            jj = 0
            for wi in ws:
                for j in range(int(J_wr[wi, ri])):
                    groups.append((int(g_off_wr[wi, ri] + j), wi, jj))
                    jj += 1
            if jj:
                call_meta.append((jj * 128, foff, ri, groups))
                foff += jj * 8   # (jj*128)/16

    # per-core wrapped int16 idx streams in call order
    idx_wrapped = []
    for c in range(NC):
        stream = idx_streams[c]
        parts = []
        for nrows, _foff, _ri, groups in call_meta:
            rows = np.concatenate(
                [stream[g * 128:(g + 1) * 128] for (g, _, _) in groups])
            blk = rows.reshape(nrows // 16, 16).T           # [16, nrows/16]
            parts.append(np.tile(blk, (8, 1)))              # [128, nrows/16]
        idx_wrapped.append(np.ascontiguousarray(np.concatenate(parts, 1)))

    # reorder dstloc columns to call order; record per-call col offset
    import ml_dtypes
    order = []
    call_coff = []
    for nrows, _foff, _ri, groups in call_meta:
        call_coff.append(len(order))
        order.extend(g for (g, _, _) in groups)
    order = np.asarray(order)
    dstloc_call = [
        np.ascontiguousarray(d[:, order]).astype(ml_dtypes.bfloat16)
        for d in dstloc_cols
    ]

    gpw = [int(J_wr[wi].sum()) for wi in range(NW)]
    return {
        "J_wr": J_wr, "NG": NG, "call_meta": call_meta, "gpw": gpw,
        "call_coff": call_coff,
        "idx_wrapped": idx_wrapped, "dstloc": dstloc_call, "total_f16": foff,
    }


def _build_program(sched):
    import concourse.bacc as bacc
    import concourse.tile as tile
    import concourse.mybir as mybir
    from concourse.masks import make_identity

    f32 = mybir.dt.float32
    bf16 = mybir.dt.bfloat16
    i16 = mybir.dt.int16
    AF = mybir.ActivationFunctionType
    OP = mybir.AluOpType
    NG = sched["NG"]
    call_meta = sched["call_meta"]
    GPW = sched["gpw"]
    CALL_COFF = sched["call_coff"]
    TOT16 = sched["total_f16"]

    nc = bacc.Bacc("TRN2", target_bir_lowering=False, debug=False,
                   num_devices=NC, num_swdge_queues=4)

    xT = nc.dram_tensor("xT", [F_IN, PERP], f32, kind="ExternalInput")
    idx = nc.dram_tensor("idx", [128, TOT16], i16, kind="ExternalInput")
    dstloc = nc.dram_tensor("dstloc", [128, NG], bf16, kind="ExternalInput")
    iotab = nc.dram_tensor("iotab", [128, 128], bf16, kind="ExternalInput")
    dinv_c = nc.dram_tensor("dinv_c", [128, NW], f32, kind="ExternalInput")
    e_meta = nc.dram_tensor("e_meta", [1, F_IN], f32, kind="ExternalInput")
    w0r = nc.dram_tensor("w0r", [1, F_IN], f32, kind="ExternalInput")
    b0T = nc.dram_tensor("b0T", [F_IN, F_IN], f32, kind="ExternalInput")
    W1 = nc.dram_tensor("W1", [F_IN, HID], f32, kind="ExternalInput")
    W2 = nc.dram_tensor("W2", [HID, HID], f32, kind="ExternalInput")
    Wl = nc.dram_tensor("Wl", [HID, N_CLS], f32, kind="ExternalInput")
    bc1b = nc.dram_tensor("bc1b", [128, HID], f32, kind="ExternalInput")
    bc2b = nc.dram_tensor("bc2b", [128, HID], f32, kind="ExternalInput")
    blb = nc.dram_tensor("blb", [128, N_CLS], f32, kind="ExternalInput")
    padm = nc.dram_tensor("padm", [128, 1], f32, kind="ExternalInput")
    out = nc.dram_tensor("out", [PERP, N_CLS], f32, kind="ExternalOutput")

    with tile.TileContext(nc) as tc:
        with (
            tc.tile_pool(name="dram", bufs=1, space="DRAM") as dram,
            tc.tile_pool(name="consts", bufs=1) as consts,
            tc.tile_pool(name="persist", bufs=1) as persist,
            tc.tile_pool(name="xs", bufs=3) as xs,
            tc.tile_pool(name="gath", bufs=6) as gpool,
            tc.tile_pool(name="oh", bufs=4) as ohp,
            tc.tile_pool(name="post", bufs=3) as post,
            tc.tile_pool(name="ps", bufs=3, space="PSUM") as ps,
            tc.tile_pool(name="ps2", bufs=3, space="PSUM") as ps2,
        ):
            ident = consts.tile([128, 128], f32)
            make_identity(nc, ident[:])
            ident64 = consts.tile([64, 64], f32)
            make_identity(nc, ident64[:])
            iota_t = consts.tile([128, 128], bf16)
            nc.sync.dma_start(iota_t[:], iotab[:])

            dinv_t = consts.tile([128, NW], f32)
            nc.sync.dma_start(dinv_t[:], dinv_c[:])
            dstloc_t = consts.tile([128, NG], bf16)
            nc.sync.dma_start(dstloc_t[:], dstloc[:])
            bc1_t = consts.tile([128, HID], f32)
            nc.sync.dma_start(bc1_t[:], bc1b[:])
            bc2_t = consts.tile([128, HID], f32)
            nc.sync.dma_start(bc2_t[:], bc2b[:])
            bl_t = consts.tile([128, N_CLS], f32)
            nc.sync.dma_start(bl_t[:], blb[:])
            padm_t = consts.tile([128, 1], f32)
            nc.sync.dma_start(padm_t[:], padm[:])
            em_t = consts.tile([1, F_IN], f32)
            nc.sync.dma_start(em_t[:], e_meta[:])
            w0_t = consts.tile([1, F_IN], f32)
            nc.sync.dma_start(w0_t[:], w0r[:])
            b0T_t = consts.tile([F_IN, F_IN], f32)
            nc.sync.dma_start(b0T_t[:], b0T[:])
            W1_t = consts.tile([F_IN, HID], f32)
            nc.sync.dma_start(W1_t[:], W1[:])
            W2_t = consts.tile([HID, HID], f32)
            nc.sync.dma_start(W2_t[:], W2[:])
            Wl_t = consts.tile([HID, N_CLS], f32)
            nc.sync.dma_start(Wl_t[:], Wl[:])

            # stage 0: W01[f_in, hid] = relu(w0@E_meta + b0) @ W1
            w0p = ps2.tile([128, 128], f32, tag="t")
            nc.tensor.matmul(w0p[:], lhsT=em_t[:], rhs=w0_t[:],
                             start=True, stop=True)
            w0s = consts.tile([128, 128], f32)
            nc.vector.tensor_add(w0s[:], w0p[:], b0T_t[:])
            nc.scalar.activation(w0s[:], w0s[:], AF.Relu)
            w01p = ps2.tile([128, HID], f32, tag="t")
            nc.tensor.matmul(w01p[:], lhsT=w0s[:], rhs=W1_t[:],
                             start=True, stop=True)
            W01_t = consts.tile([128, HID], f32)
            nc.vector.tensor_copy(W01_t[:], w01p[:])

            g1b = dram.tile([PERP, 2 * HID], bf16)
            g2b = dram.tile([PERP, 2 * HID], bf16)
            G1 = dram.tile([V, 2 * HID], bf16)
            G2 = dram.tile([V, 2 * HID], bf16)

            # stage 1: g1 (node-major) = dinv * (x @ W01), per window
            g1w = []
            for w in range(NW):
                xw = xs.tile([128, 128], f32, tag="xw")
                nc.sync.dma_start(xw[:], xT[:, w * 128:(w + 1) * 128])
                h1p = ps2.tile([HID, 128], f32, tag="t")
                nc.tensor.matmul(h1p[:], lhsT=W01_t[:], rhs=xw[:],
                                 start=True, stop=True)
                h1s = xs.tile([HID, 128], f32, tag="h1s")
                nc.vector.tensor_copy(h1s[:], h1p[:])
                h1tp = ps2.tile([128, HID], f32, tag="t")
                nc.tensor.transpose(out=h1tp[:], in_=h1s[:], identity=ident64[:])
                gw = persist.tile([128, 2 * HID], bf16, tag=f"g1_{w}")
                nc.vector.tensor_scalar_mul(gw[:, :HID], h1tp[:],
                                            dinv_t[:, w:w + 1])
                nc.vector.memset(gw[:, HID:], 0.0)
                nc.sync.dma_start(g1b[w * 128:(w + 1) * 128, :], gw[:])
                g1w.append(gw)

            nc.gpsimd.collective_compute(
                "AllGather", mybir.AluOpType.bypass,
                replica_groups=[list(range(NC))],
                ins=[g1b.opt()], outs=[G1.opt()],
            )

            import concourse.bass as bass

            def conv_pass(Gtab, gself, bc_t, out_hook):
                ci = 0
                win_groups = [[] for _ in range(NW)]
                for nrows, foff, ri, groups in call_meta:
                    J = nrows // 128
                    coff = CALL_COFF[ci]
                    idxt = xs.tile([128, nrows // 16], i16, tag="idxt")
                    nc.sync.dma_start(idxt[:], idx[:, foff:foff + nrows // 16])
                    gt = gpool.tile([128, J, 2 * HID], bf16, tag="gt")
                    nc.gpsimd.dma_gather(
                        out_ap=gt[:],
                        in_ap=Gtab[ri * RSZ:(ri + 1) * RSZ, :],
                        idxs_ap=idxt[:],
                        num_idxs=nrows,
                        num_idxs_reg=nrows,
                        elem_size=2 * HID,
                        single_packet=False,
                        queue_num=ci % 4,
                    )
                    ci += 1
                    # batched onehot for all J groups of this call
                    oh = ohp.tile([128, J * 128], bf16, tag="oh")
                    i0 = iota_t[:]
                    in0 = bass.AP(i0.tensor, i0.offset,
                                  [i0.ap[0], [0, J], i0.ap[1]])
                    d0 = dstloc_t[:, coff:coff + J]
                    in1 = bass.AP(d0.tensor, d0.offset, d0.ap + [[0, 128]])
                    nc.vector.tensor_tensor(
                        out=oh[:].rearrange("p (j f) -> p j f", f=128),
                        in0=in0, in1=in1, op=OP.is_equal,
                    )
                    for (gidx, wi, jj) in groups:
                        win_groups[wi].append((gt, oh, jj))
                        if len(win_groups[wi]) == GPW[wi]:
                            pw = ps.tile([128, HID], f32, tag="seg")
                            ng = len(win_groups[wi])
                            for k, (gt_, oh_, jj_) in enumerate(win_groups[wi]):
                                nc.tensor.matmul(
                                    pw[:],
                                    lhsT=oh_[:, jj_ * 128:(jj_ + 1) * 128],
                                    rhs=gt_[:, jj_, :HID],
                                    start=(k == 0), stop=(k == ng - 1),
                                )
                            out_hook(wi, pw, gself[wi], bc_t)
                            win_groups[wi] = []

            g2w = []

            def post1(wi, pw, gsw, bct):
                t1 = post.tile([128, HID], f32, tag="t1")
                nc.vector.tensor_add(t1[:], pw[:], gsw[:, :HID])
                nc.vector.tensor_scalar_mul(t1[:], t1[:], dinv_t[:, wi:wi + 1])
                nc.vector.tensor_add(t1[:], t1[:], bct[:])
                nc.scalar.activation(t1[:], t1[:], AF.Relu)
                if wi == NW - 1:
                    nc.vector.tensor_scalar_mul(t1[:], t1[:], padm_t[:, 0:1])
                atp = ps2.tile([HID, 128], f32, tag="t")
                nc.tensor.transpose(out=atp[:], in_=t1[:], identity=ident[:])
                ats = post.tile([HID, 128], f32, tag="a1t")
                nc.vector.tensor_copy(ats[:], atp[:])
                h2p = ps2.tile([128, HID], f32, tag="t")
                nc.tensor.matmul(h2p[:], lhsT=ats[:], rhs=W2_t[:],
                                 start=True, stop=True)
                gw2 = persist.tile([128, 2 * HID], bf16, tag=f"g2_{wi}")
                nc.vector.tensor_scalar_mul(gw2[:, :HID], h2p[:],
                                            dinv_t[:, wi:wi + 1])
                nc.vector.memset(gw2[:, HID:], 0.0)
                nc.sync.dma_start(g2b[wi * 128:(wi + 1) * 128, :], gw2[:])
                g2w.append(gw2)

            conv_pass(G1, g1w, bc1_t, post1)

            nc.gpsimd.collective_compute(
                "AllGather", mybir.AluOpType.bypass,
                replica_groups=[list(range(NC))],
                ins=[g2b.opt()], outs=[G2.opt()],
            )

            def post2(wi, pw, gsw, bct):
                t1 = post.tile([128, HID], f32, tag="t2")
                nc.vector.tensor_add(t1[:], pw[:], gsw[:, :HID])
                nc.vector.tensor_scalar_mul(t1[:], t1[:], dinv_t[:, wi:wi + 1])
                nc.vector.tensor_add(t1[:], t1[:], bct[:])
                nc.scalar.activation(t1[:], t1[:], AF.Relu)
                atp = ps2.tile([HID, 128], f32, tag="t")
                nc.tensor.transpose(out=atp[:], in_=t1[:], identity=ident[:])
                ats = post.tile([HID, 128], f32, tag="a2t")
                nc.vector.tensor_copy(ats[:], atp[:])
                lp = ps2.tile([128, N_CLS], f32, tag="t")
                nc.tensor.matmul(lp[:], lhsT=ats[:], rhs=Wl_t[:],
                                 start=True, stop=True)
                lg = post.tile([128, N_CLS], f32, tag="lgs")
                nc.vector.tensor_add(lg[:], lp[:], bl_t[:])
                mx = post.tile([128, 1], f32, tag="mx")
                nc.vector.tensor_reduce(out=mx[:], in_=lg[:],
                                        axis=mybir.AxisListType.X, op=OP.max)
                nc.vector.tensor_scalar(out=lg[:], in0=lg[:], scalar1=mx[:],
                                        scalar2=None, op0=OP.subtract)
                ex = post.tile([128, N_CLS], f32, tag="ex")
                nc.scalar.activation(ex[:], lg[:], AF.Exp)
                sm = post.tile([128, 1], f32, tag="sm")
                nc.vector.tensor_reduce(out=sm[:], in_=ex[:],
                                        axis=mybir.AxisListType.X, op=OP.add)
                nc.scalar.activation(sm[:], sm[:], AF.Ln)
                nc.vector.tensor_scalar(out=lg[:], in0=lg[:], scalar1=sm[:],
                                        scalar2=None, op0=OP.subtract)
                nc.sync.dma_start(out[wi * 128:(wi + 1) * 128, :], lg[:])

            conv_pass(G2, g2w, bc2_t, post2)

    nc.compile()
    return nc


def kernel(**inputs):
    from concourse import bass_utils

    x = np.asarray(inputs["x"], np.float32)
    ei = np.asarray(inputs["edge_index"])
    E_meta = np.asarray(inputs["E_meta"], np.float32)
    w0 = np.asarray(inputs["w0"], np.float32)
    b0 = np.asarray(inputs["b0"], np.float32)
    W1 = np.asarray(inputs["W1"], np.float32)
    bc1 = np.asarray(inputs["bc1"], np.float32)
    W2 = np.asarray(inputs["W2"], np.float32)
    bc2 = np.asarray(inputs["bc2"], np.float32)
    Wl = np.asarray(inputs["Wl"], np.float32)
    bl = np.asarray(inputs["bl"], np.float32)

    src = ei[0].astype(np.int64)
    dst = ei[1].astype(np.int64)

    sched = _build_schedule(src, dst)
    key = (sched["NG"], sched["total_f16"],
           tuple(int(v) for v in sched["J_wr"].flatten()))
    if key not in _cache:
        _cache[key] = _build_program(sched)
    nc = _cache[key]

    deg = 1.0 + np.bincount(dst, minlength=N).astype(np.float32)
    dinv = (1.0 / np.sqrt(deg)).astype(np.float32)

    in_maps = []
    for c in range(NC):
        xTc = np.zeros((F_IN, PERP), np.float32)
        xTc[:, :PER] = x[c * PER:(c + 1) * PER].T
        dv = np.ones(PERP, np.float32)
        dv[:PER] = dinv[c * PER:(c + 1) * PER]
        in_maps.append({
            "xT": xTc,
            "idx": sched["idx_wrapped"][c],
            "dstloc": sched["dstloc"][c],
            "iotab": _iotab(),
            "dinv_c": np.ascontiguousarray(dv.reshape(NW, 128).T),
            "e_meta": E_meta,
            "w0r": np.ascontiguousarray(w0.reshape(1, F_IN)),
            "b0T": np.ascontiguousarray(b0.T),
            "W1": W1, "W2": W2, "Wl": Wl,
            "bc1b": np.tile(bc1.reshape(1, HID), (128, 1)),
            "bc2b": np.tile(bc2.reshape(1, HID), (128, 1)),
            "blb": np.tile(bl.reshape(1, N_CLS), (128, 1)),
            "padm": np.concatenate([np.ones((84, 1), np.float32),
                                    np.zeros((44, 1), np.float32)]),
        })

    res = bass_utils.run_bass_kernel_spmd(
        nc, in_maps, core_ids=list(range(NC)),
        trace=bool(int(os.environ.get("GCN_TRACE", "0"))),
    )
    kernel.last_exec_time_ns = res.exec_time_ns

    outp = np.empty((N, N_CLS), np.float32)
    for c in range(NC):
        outp[c * PER:(c + 1) * PER] = res.results[c]["out"][:PER]
    return outp


# revision 6
# speedup vs baseline: 1.5493x; 1.0871x over previous
"""GCN (2-layer, symmetric-norm, self-loops) on 8 TRN2 NeuronCores.

Strategy:
  - Nodes dst-sharded across 8 cores (12500 each, padded to 12544 = 98*128).
  - Per conv: h@W computed per core; g = dinv*h AllGathered into a full
    replicated table [100352, 64] f32 in each core's DRAM.
  - Per-edge message gather via gpsimd dma_gather (int16 indices -> 4
    source ranges of 25088 rows), edge-major groups of 128.
  - Segment-sum via one-hot matmul: onehot[128 edges, 128 dst] built on DVE
    (iota vs dst_local compare), accumulated in PSUM per 128-node window.
  - Self-loop term, bias, relu, next-layer matmuls, log_softmax on device.
  - Host does schedule building only: edge sorting/padding, gather index
    construction, degree bincount for normalization, input layout.
  - The group structure (counts per window/range) is max-pooled over cores
    so all 8 cores run the same SPMD program; per-core data (indices,
    dst-locals, dinv, x) differ via in_maps.
"""

import os
import sys
import numpy as np

sys.path.insert(0, "/opt/trn_rl_repo")

N = 100000
NC = 8
PER = 12500          # real nodes per core
PERP = 12544         # padded (98 * 128)
NW = PERP // 128     # 98 windows
V = NC * PERP        # 100352 table rows
NRANGE = 4
RSZ = V // NRANGE    # 25088 (= 2 shards), < 32768 so int16 works
F_IN = 128
HID = 64
N_CLS = 40
SBW = 4              # windows per superblock (gather call granularity)

_cache = {}


def _iotab():
    import ml_dtypes
    return np.tile(np.arange(128, dtype=np.float32), (128, 1)).astype(
        ml_dtypes.bfloat16)


def _build_schedule(src, dst):
    """Host schedule: uniform-across-cores group structure, per-core data."""
    g_row = (src // PER) * PERP + (src % PER)   # gather-table row of src node
    core = dst // PER
    dstl = dst - core * PER                     # local dst 0..12499
    w = dstl // 128
    r = g_row // RSZ
    loc = (g_row % RSZ).astype(np.int16)        # in-range row, 0..25087
    wloc = (dstl % 128).astype(np.float32)

    key = (core * NW + w) * NRANGE + r
    order = np.argsort(key, kind="stable")
    key_s = key[order]
    loc_s = loc[order]
    wloc_s = wloc[order]
    cell_ids, cell_counts = np.unique(key_s, return_counts=True)
    counts_flat = np.zeros(NC * NW * NRANGE, np.int64)
    counts_flat[cell_ids] = cell_counts
    counts = counts_flat.reshape(NC, NW, NRANGE)

    # uniform group counts: max over cores per (w, r)
    J_wr = np.ceil(counts.max(axis=0) / 128).astype(np.int64)   # [NW, NRANGE]
    J_wr = np.maximum(J_wr, 1)
    NG = int(J_wr.sum())

    dummy_loc = np.int16(PER)   # per-range zero pad row (shard-local row 12500)

    cell_starts = np.zeros(NC * NW * NRANGE + 1, np.int64)
    np.cumsum(counts_flat, out=cell_starts[1:])

    idx_streams = []
    dstloc_cols = []
    for c in range(NC):
        idx_parts = []
        dl_parts = []
        for wi in range(NW):
            for ri in range(NRANGE):
                cid = (c * NW + wi) * NRANGE + ri
                s0, s1 = cell_starts[cid], cell_starts[cid + 1]
                n_real = s1 - s0
                n_pad = int(J_wr[wi, ri]) * 128
                li = np.full(n_pad, dummy_loc, np.int16)
                li[:n_real] = loc_s[s0:s1]
                dl = np.zeros(n_pad, np.float32)
                dl[:n_real] = wloc_s[s0:s1]
                idx_parts.append(li)
                dl_parts.append(dl)
        idx_all = np.concatenate(idx_parts)
        dl_all = np.concatenate(dl_parts)
        dstloc_cols.append(dl_all.reshape(NG, 128).T)   # (w,r,j) order for now
        idx_streams.append(idx_all)

    # group-index offsets in (w, r, j) order
    g_off_wr = np.zeros((NW, NRANGE), np.int64)
    t = 0
    for wi in range(NW):
        for ri in range(NRANGE):
            g_off_wr[wi, ri] = t
            t += int(J_wr[wi, ri])

    # gather calls: per (superblock, range)
    n_sb = (NW + SBW - 1) // SBW
    call_meta = []   # (nrows, free16_off, range, [(gidx, window, col_in_call)])
    foff = 0
    for sb in range(n_sb):
        ws = range(sb * SBW, min((sb + 1) * SBW, NW))
        for ri in range(NRANGE):
            groups = []
            jj = 0
            for wi in ws:
                for j in range(int(J_wr[wi, ri])):
                    groups.append((int(g_off_wr[wi, ri] + j), wi, jj))
                    jj += 1
            if jj:
                call_meta.append((jj * 128, foff, ri, groups))
                foff += jj * 8   # (jj*128)/16

    # per-core wrapped int16 idx streams in call order
    idx_wrapped = []
    for c in range(NC):
        stream = idx_streams[c]
        parts = []
        for nrows, _foff, _ri, groups in call_meta:
            rows = np.concatenate(
                [stream[g * 128:(g + 1) * 128] for (g, _, _) in groups])
            blk = rows.reshape(nrows // 16, 16).T           # [16, nrows/16]
            parts.append(np.tile(blk, (8, 1)))              # [128, nrows/16]
        idx_wrapped.append(np.ascontiguousarray(np.concatenate(parts, 1)))

    # reorder dstloc columns to call order; record per-call col offset
    import ml_dtypes
    order = []
    call_coff = []
    for nrows, _foff, _ri, groups in call_meta:
        call_coff.append(len(order))
        order.extend(g for (g, _, _) in groups)
    order = np.asarray(order)
    dstloc_call = [
        np.ascontiguousarray(d[:, order]).astype(ml_dtypes.bfloat16)
        for d in dstloc_cols
    ]

    gpw = [int(J_wr[wi].sum()) for wi in range(NW)]
    return {
        "J_wr": J_wr, "NG": NG, "call_meta": call_meta, "gpw": gpw,
        "call_coff": call_coff,
        "idx_wrapped": idx_wrapped, "dstloc": dstloc_call, "total_f16": foff,
    }


def _build_program(sched):
    import concourse.bacc as bacc
    import concourse.tile as tile
    import concourse.mybir as mybir
    from concourse.masks import make_identity

    f32 = mybir.dt.float32
    bf16 = mybir.dt.bfloat16
    i16 = mybir.dt.int16
    AF = mybir.ActivationFunctionType
    OP = mybir.AluOpType
    NG = sched["NG"]
    call_meta = sched["call_meta"]
    GPW = sched["gpw"]
    CALL_COFF = sched["call_coff"]
    TOT16 = sched["total_f16"]

    nc = bacc.Bacc("TRN2", target_bir_lowering=False, debug=False,
                   num_devices=NC, num_swdge_queues=4)

    xT = nc.dram_tensor("xT", [F_IN, PERP], f32, kind="ExternalInput")
    idx = nc.dram_tensor("idx", [128, TOT16], i16, kind="ExternalInput")
    dstloc = nc.dram_tensor("dstloc", [128, NG], bf16, kind="ExternalInput")
    iotab = nc.dram_tensor("iotab", [128, 128], bf16, kind="ExternalInput")
    dinv_c = nc.dram_tensor("dinv_c", [128, NW], f32, kind="ExternalInput")
    e_meta = nc.dram_tensor("e_meta", [1, F_IN], f32, kind="ExternalInput")
    w0r = nc.dram_tensor("w0r", [1, F_IN], f32, kind="ExternalInput")
    b0T = nc.dram_tensor("b0T", [F_IN, F_IN], f32, kind="ExternalInput")
    W1 = nc.dram_tensor("W1", [F_IN, HID], f32, kind="ExternalInput")
    W2 = nc.dram_tensor("W2", [HID, HID], f32, kind="ExternalInput")
    Wl = nc.dram_tensor("Wl", [HID, N_CLS], f32, kind="ExternalInput")
    bc1b = nc.dram_tensor("bc1b", [128, HID], f32, kind="ExternalInput")
    bc2b = nc.dram_tensor("bc2b", [128, HID], f32, kind="ExternalInput")
    blb = nc.dram_tensor("blb", [128, N_CLS], f32, kind="ExternalInput")
    padm = nc.dram_tensor("padm", [128, 1], f32, kind="ExternalInput")
    out = nc.dram_tensor("out", [PERP, N_CLS], f32, kind="ExternalOutput")

    with tile.TileContext(nc) as tc:
        with (
            tc.tile_pool(name="dram", bufs=1, space="DRAM") as dram,
            tc.tile_pool(name="consts", bufs=1) as consts,
            tc.tile_pool(name="persist", bufs=1) as persist,
            tc.tile_pool(name="xs", bufs=4) as xs,
            tc.tile_pool(name="gath", bufs=8) as gpool,
            tc.tile_pool(name="oh", bufs=4) as ohp,
            tc.tile_pool(name="post", bufs=3) as post,
            tc.tile_pool(name="ps", bufs=4, space="PSUM") as ps,
            tc.tile_pool(name="ps2", bufs=4, space="PSUM") as ps2,
        ):
            ident = consts.tile([128, 128], f32)
            make_identity(nc, ident[:])
            ident64 = consts.tile([64, 64], f32)
            make_identity(nc, ident64[:])
            iota_t = consts.tile([128, 128], bf16)
            nc.sync.dma_start(iota_t[:], iotab[:])

            dinv_t = consts.tile([128, NW], f32)
            nc.sync.dma_start(dinv_t[:], dinv_c[:])
            dstloc_t = consts.tile([128, NG], bf16)
            nc.sync.dma_start(dstloc_t[:], dstloc[:])
            bc1_t = consts.tile([128, HID], f32)
            nc.sync.dma_start(bc1_t[:], bc1b[:])
            bc2_t = consts.tile([128, HID], f32)
            nc.sync.dma_start(bc2_t[:], bc2b[:])
            bl_t = consts.tile([128, N_CLS], f32)
            nc.sync.dma_start(bl_t[:], blb[:])
            padm_t = consts.tile([128, 1], f32)
            nc.sync.dma_start(padm_t[:], padm[:])
            em_t = consts.tile([1, F_IN], f32)
            nc.sync.dma_start(em_t[:], e_meta[:])
            w0_t = consts.tile([1, F_IN], f32)
            nc.sync.dma_start(w0_t[:], w0r[:])
            b0T_t = consts.tile([F_IN, F_IN], f32)
            nc.sync.dma_start(b0T_t[:], b0T[:])
            W1_t = consts.tile([F_IN, HID], f32)
            nc.sync.dma_start(W1_t[:], W1[:])
            W2_t = consts.tile([HID, HID], f32)
            nc.sync.dma_start(W2_t[:], W2[:])
            Wl_t = consts.tile([HID, N_CLS], f32)
            nc.sync.dma_start(Wl_t[:], Wl[:])

            # stage 0: W01[f_in, hid] = relu(w0@E_meta + b0) @ W1
            w0p = ps2.tile([128, 128], f32, tag="t")
            nc.tensor.matmul(w0p[:], lhsT=em_t[:], rhs=w0_t[:],
                             start=True, stop=True)
            w0s = consts.tile([128, 128], f32)
            nc.vector.tensor_add(w0s[:], w0p[:], b0T_t[:])
            nc.scalar.activation(w0s[:], w0s[:], AF.Relu)
            w01p = ps2.tile([128, HID], f32, tag="t")
            nc.tensor.matmul(w01p[:], lhsT=w0s[:], rhs=W1_t[:],
                             start=True, stop=True)
            W01_t = consts.tile([128, HID], f32)
            nc.vector.tensor_copy(W01_t[:], w01p[:])

            g1b = dram.tile([PERP, 2 * HID], bf16)
            g2b = dram.tile([PERP, 2 * HID], bf16)
            G1 = dram.tile([V, 2 * HID], bf16)
            G2 = dram.tile([V, 2 * HID], bf16)

            # stage 1: g1 (node-major) = dinv * (x @ W01), per window
            g1w = []
            for w in range(NW):
                xw = xs.tile([128, 128], f32, tag="xw")
                nc.sync.dma_start(xw[:], xT[:, w * 128:(w + 1) * 128])
                h1p = ps2.tile([HID, 128], f32, tag="t")
                nc.tensor.matmul(h1p[:], lhsT=W01_t[:], rhs=xw[:],
                                 start=True, stop=True)
                h1s = xs.tile([HID, 128], f32, tag="h1s")
                nc.vector.tensor_copy(h1s[:], h1p[:])
                h1tp = ps2.tile([128, HID], f32, tag="t")
                nc.tensor.transpose(out=h1tp[:], in_=h1s[:], identity=ident64[:])
                gw = persist.tile([128, 2 * HID], bf16, tag=f"g1_{w}")
                nc.vector.tensor_scalar_mul(gw[:, :HID], h1tp[:],
                                            dinv_t[:, w:w + 1])
                nc.vector.memset(gw[:, HID:], 0.0)
                nc.scalar.dma_start(g1b[w * 128:(w + 1) * 128, :], gw[:])
                g1w.append(gw)

            nc.gpsimd.collective_compute(
                "AllGather", mybir.AluOpType.bypass,
                replica_groups=[list(range(NC))],
                ins=[g1b.opt()], outs=[G1.opt()],
            )

            import concourse.bass as bass

            def conv_pass(Gtab, gself, bc_t, out_hook):
                ci = 0
                win_groups = [[] for _ in range(NW)]
                for nrows, foff, ri, groups in call_meta:
                    J = nrows // 128
                    coff = CALL_COFF[ci]
                    idxt = xs.tile([128, nrows // 16], i16, tag="idxt")
                    nc.sync.dma_start(idxt[:], idx[:, foff:foff + nrows // 16])
                    gt = gpool.tile([128, J, 2 * HID], bf16, tag="gt")
                    nc.gpsimd.dma_gather(
                        out_ap=gt[:],
                        in_ap=Gtab[ri * RSZ:(ri + 1) * RSZ, :],
                        idxs_ap=idxt[:],
                        num_idxs=nrows,
                        num_idxs_reg=nrows,
                        elem_size=2 * HID,
                        single_packet=False,
                        queue_num=ci % 4,
                    )
                    ci += 1
                    # batched onehot for all J groups of this call
                    oh = ohp.tile([128, J * 128], bf16, tag="oh")
                    i0 = iota_t[:]
                    in0 = bass.AP(i0.tensor, i0.offset,
                                  [i0.ap[0], [0, J], i0.ap[1]])
                    d0 = dstloc_t[:, coff:coff + J]
                    in1 = bass.AP(d0.tensor, d0.offset, d0.ap + [[0, 128]])
                    nc.vector.tensor_tensor(
                        out=oh[:].rearrange("p (j f) -> p j f", f=128),
                        in0=in0, in1=in1, op=OP.is_equal,
                    )
                    for (gidx, wi, jj) in groups:
                        win_groups[wi].append((gt, oh, jj))
                        if len(win_groups[wi]) == GPW[wi]:
                            pw = ps.tile([128, HID], f32, tag="seg")
                            ng = len(win_groups[wi])
                            for k, (gt_, oh_, jj_) in enumerate(win_groups[wi]):
                                nc.tensor.matmul(
                                    pw[:],
                                    lhsT=oh_[:, jj_ * 128:(jj_ + 1) * 128],
                                    rhs=gt_[:, jj_, :HID],
                                    start=(k == 0), stop=(k == ng - 1),
                                )
                            out_hook(wi, pw, gself[wi], bc_t)
                            win_groups[wi] = []

            g2w = []

            def post1(wi, pw, gsw, bct):
                t1 = post.tile([128, HID], f32, tag="t1")
                nc.vector.tensor_add(t1[:], pw[:], gsw[:, :HID])
                nc.vector.tensor_scalar_mul(t1[:], t1[:], dinv_t[:, wi:wi + 1])
                nc.vector.tensor_add(t1[:], t1[:], bct[:])
                nc.scalar.activation(t1[:], t1[:], AF.Relu)
                if wi == NW - 1:
                    nc.vector.tensor_scalar_mul(t1[:], t1[:], padm_t[:, 0:1])
                atp = ps2.tile([HID, 128], f32, tag="t")
                nc.tensor.transpose(out=atp[:], in_=t1[:], identity=ident[:])
                ats = post.tile([HID, 128], f32, tag="a1t")
                nc.vector.tensor_copy(ats[:], atp[:])
                h2p = ps2.tile([128, HID], f32, tag="t")
                nc.tensor.matmul(h2p[:], lhsT=ats[:], rhs=W2_t[:],
                                 start=True, stop=True)
                gw2 = persist.tile([128, 2 * HID], bf16, tag=f"g2_{wi}")
                nc.vector.tensor_scalar_mul(gw2[:, :HID], h2p[:],
                                            dinv_t[:, wi:wi + 1])
                nc.vector.memset(gw2[:, HID:], 0.0)
                nc.scalar.dma_start(g2b[wi * 128:(wi + 1) * 128, :], gw2[:])
                g2w.append(gw2)

            conv_pass(G1, g1w, bc1_t, post1)

            nc.gpsimd.collective_compute(
                "AllGather", mybir.AluOpType.bypass,
                replica_groups=[list(range(NC))],
                ins=[g2b.opt()], outs=[G2.opt()],
            )

            def post2(wi, pw, gsw, bct):
                t1 = post.tile([128, HID], f32, tag="t2")
                nc.vector.tensor_add(t1[:], pw[:], gsw[:, :HID])
                nc.vector.tensor_scalar_mul(t1[:], t1[:], dinv_t[:, wi:wi + 1])
                nc.vector.tensor_add(t1[:], t1[:], bct[:])
                nc.scalar.activation(t1[:], t1[:], AF.Relu)
                atp = ps2.tile([HID, 128], f32, tag="t")
                nc.tensor.transpose(out=atp[:], in_=t1[:], identity=ident[:])
                ats = post.tile([HID, 128], f32, tag="a2t")
                nc.vector.tensor_copy(ats[:], atp[:])
                lp = ps2.tile([128, N_CLS], f32, tag="t")
                nc.tensor.matmul(lp[:], lhsT=ats[:], rhs=Wl_t[:],
                                 start=True, stop=True)
                lg = post.tile([128, N_CLS], f32, tag="lgs")
                nc.vector.tensor_add(lg[:], lp[:], bl_t[:])
                mx = post.tile([128, 1], f32, tag="mx")
                nc.vector.tensor_reduce(out=mx[:], in_=lg[:],
                                        axis=mybir.AxisListType.X, op=OP.max)
                nc.vector.tensor_scalar(out=lg[:], in0=lg[:], scalar1=mx[:],
                                        scalar2=None, op0=OP.subtract)
                ex = post.tile([128, N_CLS], f32, tag="ex")
                nc.scalar.activation(ex[:], lg[:], AF.Exp)
                sm = post.tile([128, 1], f32, tag="sm")
                nc.vector.tensor_reduce(out=sm[:], in_=ex[:],
                                        axis=mybir.AxisListType.X, op=OP.add)
                nc.scalar.activation(sm[:], sm[:], AF.Ln)
                nc.vector.tensor_scalar(out=lg[:], in0=lg[:], scalar1=sm[:],
                                        scalar2=None, op0=OP.subtract)
                nc.scalar.dma_start(out[wi * 128:(wi + 1) * 128, :], lg[:])

            conv_pass(G2, g2w, bc2_t, post2)

    nc.compile()
    return nc


def kernel(**inputs):
    from concourse import bass_utils

    x = np.asarray(inputs["x"], np.float32)
    ei = np.asarray(inputs["edge_index"])
    E_meta = np.asarray(inputs["E_meta"], np.float32)
    w0 = np.asarray(inputs["w0"], np.float32)
    b0 = np.asarray(inputs["b0"], np.float32)
    W1 = np.asarray(inputs["W1"], np.float32)
    bc1 = np.asarray(inputs["bc1"], np.float32)
    W2 = np.asarray(inputs["W2"], np.float32)
    bc2 = np.asarray(inputs["bc2"], np.float32)
    Wl = np.asarray(inputs["Wl"], np.float32)
    bl = np.asarray(inputs["bl"], np.float32)

    src = ei[0].astype(np.int64)
    dst = ei[1].astype(np.int64)

    sched = _build_schedule(src, dst)
    key = (sched["NG"], sched["total_f16"],
           tuple(int(v) for v in sched["J_wr"].flatten()))
    if key not in _cache:
        _cache[key] = _build_program(sched)
    nc = _cache[key]

    deg = 1.0 + np.bincount(dst, minlength=N).astype(np.float32)
    dinv = (1.0 / np.sqrt(deg)).astype(np.float32)

    in_maps = []
    for c in range(NC):
        xTc = np.zeros((F_IN, PERP), np.float32)
        xTc[:, :PER] = x[c * PER:(c + 1) * PER].T
        dv = np.ones(PERP, np.float32)
        dv[:PER] = dinv[c * PER:(c + 1) * PER]
        in_maps.append({
            "xT": xTc,
            "idx": sched["idx_wrapped"][c],
            "dstloc": sched["dstloc"][c],
            "iotab": _iotab(),
            "dinv_c": np.ascontiguousarray(dv.reshape(NW, 128).T),
            "e_meta": E_meta,
            "w0r": np.ascontiguousarray(w0.reshape(1, F_IN)),
            "b0T": np.ascontiguousarray(b0.T),
            "W1": W1, "W2": W2, "Wl": Wl,
            "bc1b": np.tile(bc1.reshape(1, HID), (128, 1)),
            "bc2b": np.tile(bc2.reshape(1, HID), (128, 1)),
            "blb": np.tile(bl.reshape(1, N_CLS), (128, 1)),
            "padm": np.concatenate([np.ones((84, 1), np.float32),
                                    np.zeros((44, 1), np.float32)]),
        })

    res = bass_utils.run_bass_kernel_spmd(
        nc, in_maps, core_ids=list(range(NC)),
        trace=bool(int(os.environ.get("GCN_TRACE", "0"))),
    )
    kernel.last_exec_time_ns = res.exec_time_ns

    outp = np.empty((N, N_CLS), np.float32)
    for c in range(NC):
        outp[c * PER:(c + 1) * PER] = res.results[c]["out"][:PER]
    return outp
